# revision 57
# baseline (speedup 1.0000x reference)
"""Trainium2 Bass kernel for LongcatFlash MoE experts (expert-parallel, 8 cores).

Problem: T=4096 tokens, H=1024, I=512, 32 routed + 8 zero (identity) experts,
top-4 routing, per-expert capacity 768.

Strategy (sharding_hint = expert parallelism):
  - Host: compute routing (stable sort by expert, capacity clip), permute
    tokens to their expert's core (the "all-to-all"), build per-core packed
    activation buffers with tokens on the GEMM free dimension.
  - Device (8 cores, SPMD): each core owns 4 routed experts; per expert run
    the gated MLP as tiled matmuls with tokens on the free dim:
        gu[o, c]  = sum_h guT[h, o] * xT[h, c]      (o = 2I rows, c = tokens)
        mid[i, c] = silu(gate[i, c]) * up[i, c]
        y[h, c]   = sum_i dnT[i, h] * mid[i, c]
  - Host: gather per-assignment outputs, scale by router weight, scatter-add
    back per token, add the zero-expert weighted-identity term.

Precision modes:
  - "fp8" (default): e4m3 weights/activations, DoubleRow matmuls (K=256 per
    MM, 2 fp8 MACs per PE cell per cycle).  Weights are pre-scaled by 128 on
    the host to sit in e4m3's precision sweet spot; the 1/128 is folded into
    the on-device silu/up scaling and the host-side combine.  TRN's e4m3 has
    max +-240 (= ml_dtypes.float8_e4m3, not the OCP "fn" variant).
  - "bf16": fallback, plain K=128 matmuls.
"""

import math
import os

import numpy as np

N_CORES = 8
R = 32  # routed experts
E_PER_CORE = R // N_CORES  # 4
CAPACITY = 768
H = 1024
I_DIM = 512
HT = H // 128  # 8 h-tiles
OT = 2 * I_DIM // 128  # 8 o-tiles of gate_up
IT = I_DIM // 128  # 4 i-tiles

WSCALE = 128.0  # fp8 weight pre-scale (power of 2: exact to undo)

PREC = os.environ.get("MOE_PREC", "fp8")

LAST_RUN = {}  # filled with exec_time_ns etc. for test harness use


def _route(idx, wts, n_tok):
    """Replicates the reference's capacity-buffer routing exactly.

    Returns per-assignment (expert, token, weight, slot, flat_index) for kept
    routed assignments, sorted by expert (stable), plus zero-expert weights.
    """
    K = idx.shape[1]
    A = n_tok * K
    flat_e = idx.reshape(-1).astype(np.int64)
    flat_t = np.repeat(np.arange(n_tok, dtype=np.int64), K)
    flat_w = wts.reshape(-1)
    order = np.argsort(flat_e, kind="stable")
    se = flat_e[order]
    st = flat_t[order]
    sw = flat_w[order]
    counts = np.bincount(flat_e, minlength=R + 8)
    starts = np.cumsum(counts) - counts
    pos = np.arange(A, dtype=np.int64) - starts[se]
    valid = (se < R) & (pos < CAPACITY)
    zero_w = np.where(idx >= R, wts, 0.0).sum(axis=1)
    return (
        se[valid],
        st[valid],
        sw[valid],
        pos[valid],
        order[valid],
        zero_w,
    )


def _chunks(S):
    n = (S + 511) // 512
    base = S // n
    rem = S - base * n
    out = []
    c0 = 0
    for i in range(n):
        cn = base + (1 if i < rem else 0)
        out.append((c0, cn))
        c0 += cn
    return out


_BUILD_CACHE = {}


def _build_fp8(SL):
    """fp8 e4m3 DoubleRow pipeline: per expert-slot
      4 gate psums + 4 up psums (4 DoubleRow MMs each, K=256),
      silu+mult -> fp8 mid, 8 down psums (2 DoubleRow MMs each),
      copy -> bf16 y, DMA out per h-pair.

    SL is a tuple of per-slot free-dim sizes (descending).  Every core runs
    the same instruction stream, but expert-to-slot assignment is per-core
    (rank k*8+c goes to core c slot k), so slot k only needs to cover the
    k-th largest per-core expert count.
    """
    import concourse.bacc as bacc
    import concourse.bass as bass
    import concourse.mybir as mybir
    from concourse import tile

    key = (tuple(SL), "fp8")
    if key in _BUILD_CACHE:
        return _BUILD_CACHE[key]

    FT = mybir.dt.float32
    F8 = mybir.dt.float8e4
    BF = mybir.dt.bfloat16
    DR = mybir.MatmulPerfMode.DoubleRow
    silu_fn = mybir.ActivationFunctionType.Silu

    assert all(256 <= s <= 512 for s in SL), "fp8 path assumes 256 <= S <= 512"

    nc = bacc.Bacc(None)
    xts = [
        nc.declare_dram_parameter(f"xt{k}", [128, HT, SL[k]], F8, isOutput=False)
        for k in range(E_PER_CORE)
    ]
    gu_d = nc.declare_dram_parameter("guw", [E_PER_CORE, 128, HT, 1024], F8, isOutput=False)
    dn_d = nc.declare_dram_parameter("dnw", [E_PER_CORE, 128, IT, 1024], F8, isOutput=False)
    yts = [
        nc.declare_dram_parameter(f"yt{k}", [128, HT, SL[k]], BF, isOutput=True)
        for k in range(E_PER_CORE)
    ]

    inv = 1.0 / WSCALE

    with tile.TileContext(nc) as tc:
        with (
            tc.tile_pool(name="xpool", bufs=3) as xpool,
            tc.tile_pool(name="gupool", bufs=3) as gupool,
            tc.tile_pool(name="dnpool", bufs=3) as dnpool,
            tc.tile_pool(name="midpool", bufs=2) as midpool,
            # sil tiles are ACT-written; unique slots (no reuse) keep the
            # Activation instruction at a single sync-wait (AC struct limit 1)
            tc.tile_pool(name="silpool", bufs=E_PER_CORE * IT) as silpool,
            tc.tile_pool(name="ypool", bufs=3) as ypool,
            tc.tile_pool(name="pgpool", bufs=2, space="PSUM") as pgpool,
            tc.tile_pool(name="pupool", bufs=2, space="PSUM") as pupool,
            tc.tile_pool(name="pypool", bufs=4, space="PSUM") as pypool,
        ):
            # Warm the PE clock gate (HAM) with dummy matmuls during the
            # initial DMA wait: ~3us of sustained PE activity flips the clock
            # from 1.2 to 2.4 GHz, so the real stream starts at full rate.
            wz = ypool.tile([128, 2, 256], F8, tag="warm")
            nc.vector.memset(wz[:], 0)
            pd = pypool.tile([128, SL[0]], FT, tag="py")
            for _ in range(16):
                nc.tensor.matmul(
                    pd[:, 0:256], wz[:, :, 0:128], wz[:], start=True, stop=True,
                    perf_mode=DR,
                )
            # Just-in-time per-expert h-pair loads (upfront whole-tensor
            # prefetch saturates the DMA fabric and starves the critical
            # expert-0 pieces).  gu/dn ride the sync (SP HWDGE) ring, x the
            # scalar (ACT HWDGE) ring.  Expert 0's first gu pair is split
            # gate-half/up-half so the first matmul unblocks earlier.
            for e in range(E_PER_CORE):
                Se = SL[e]
                xt_d = xts[e]
                yt_d = yts[e]
                xe = xpool.tile([128, HT, Se], F8, tag="xt")
                ge = gupool.tile([128, HT, 1024], F8, tag="gu")
                de = dnpool.tile([128, IT, 1024], F8, tag="dn")
                # expert 0 is latency-critical: fine pieces so the first MMs
                # unblock ASAP, dn0 on the scalar ring right after x0 (ready
                # ~13us, needed ~17).  Experts 1-3 are prefetched a full
                # phase ahead, so coarse 512KB transfers (fewer ~1.3us HWDGE
                # ring round trips, better DMA efficiency) win there.
                if e == 0:
                    nc.sync.dma_start(ge[:, 0:2, 0:512], gu_d[0, :, 0:2, 0:512])
                    nc.scalar.dma_start(xe[:, 0:2, :], xt_d[:, 0:2, :])
                    nc.sync.dma_start(ge[:, 0:2, 512:1024], gu_d[0, :, 0:2, 512:1024])
                    for hh in range(1, HT // 2):
                        nc.sync.dma_start(
                            ge[:, 2 * hh : 2 * hh + 2, :],
                            gu_d[0, :, 2 * hh : 2 * hh + 2, :],
                        )
                        nc.scalar.dma_start(
                            xe[:, 2 * hh : 2 * hh + 2, :], xt_d[:, 2 * hh : 2 * hh + 2, :]
                        )
                    # dn0 rides the tail of the scalar ring: sync then carries
                    # only the 5 gu pieces (each ~1.3us earlier), and dn0
                    # still lands ~1.5us before the first down matmul
                    for ii in range(IT // 2):
                        nc.scalar.dma_start(
                            de[:, 2 * ii : 2 * ii + 2, :],
                            dn_d[0, :, 2 * ii : 2 * ii + 2, :],
                        )
                else:
                    # all on the sync ring: its FIFO is program order, so the
                    # loads self-throttle behind expert-0's critical pieces,
                    # and the scalar queue stays compute-only (x triggers were
                    # firing ~1us late behind the previous expert's silu/muls)
                    nc.sync.dma_start(ge[:, 0:4, :], gu_d[e, :, 0:4, :])
                    nc.sync.dma_start(xe[:], xt_d[:])
                    nc.sync.dma_start(ge[:, 4:HT, :], gu_d[e, :, 4:HT, :])
                    nc.sync.dma_start(de[:], dn_d[e])

                mid = midpool.tile([128, IT, Se], F8, tag="mid")
                for oi in range(IT):
                    pg = pgpool.tile([128, Se], FT, tag="pg")
                    pu = pupool.tile([128, Se], FT, tag="pu")
                    for hh in range(HT // 2):
                        nc.tensor.matmul(
                            pg[:],
                            ge[:, 2 * hh : 2 * hh + 2, oi * 128 : (oi + 1) * 128],
                            xe[:, 2 * hh : 2 * hh + 2, :],
                            start=(hh == 0),
                            stop=(hh == HT // 2 - 1),
                            perf_mode=DR,
                        )
                    for hh in range(HT // 2):
                        nc.tensor.matmul(
                            pu[:],
                            ge[:, 2 * hh : 2 * hh + 2, 512 + oi * 128 : 512 + (oi + 1) * 128],
                            xe[:, 2 * hh : 2 * hh + 2, :],
                            start=(hh == 0),
                            stop=(hh == HT // 2 - 1),
                            perf_mode=DR,
                        )
                    sil = silpool.tile([128, Se], FT, tag="sil")
                    nc.scalar.activation(sil[:], pg[:], silu_fn, scale=inv)
                    nc.vector.scalar_tensor_tensor(
                        mid[:, oi, :], pu[:], inv, sil[:],
                        mybir.AluOpType.mult, mybir.AluOpType.mult,
                    )

                y3 = ypool.tile([128, HT, Se], BF, tag="yo")
                last_e = e == E_PER_CORE - 1
                for h in range(HT):
                    if last_e and h == HT - 1:
                        # pipeline the tail: the final h-tile runs as two
                        # free-dim chunks so copy/DMA of chunk 0 overlaps the
                        # matmuls of chunk 1, and the last transfer is small
                        for c0, cn in ((0, 256), (256, Se - 256)):
                            py = pypool.tile([128, cn], FT, tag="py")
                            for ii in range(IT // 2):
                                nc.tensor.matmul(
                                    py[:],
                                    de[:, 2 * ii : 2 * ii + 2, h * 128 : (h + 1) * 128],
                                    mid[:, 2 * ii : 2 * ii + 2, c0 : c0 + cn],
                                    start=(ii == 0),
                                    stop=(ii == IT // 2 - 1),
                                    perf_mode=DR,
                                )
                            if c0 == 0:
                                nc.vector.tensor_copy(y3[:, h, c0 : c0 + cn], py[:])
                                # sync ring is idle at the tail; gpsimd's Q7 is
                                # still serializing the h1-h6 write-out triggers
                                nc.sync.dma_start(
                                    yt_d[:, h, c0 : c0 + cn], y3[:, h, c0 : c0 + cn]
                                )
                            else:
                                nc.scalar.mul(y3[:, h, c0 : c0 + cn], py[:], 1.0)
                                nc.scalar.dma_start(
                                    yt_d[:, h, c0 : c0 + cn], y3[:, h, c0 : c0 + cn]
                                )
                        continue
                    py = pypool.tile([128, Se], FT, tag="py")
                    for ii in range(IT // 2):
                        nc.tensor.matmul(
                            py[:],
                            de[:, 2 * ii : 2 * ii + 2, h * 128 : (h + 1) * 128],
                            mid[:, 2 * ii : 2 * ii + 2, :],
                            start=(ii == 0),
                            stop=(ii == IT // 2 - 1),
                            perf_mode=DR,
                        )
                    # split psum->bf16 copies across DVE and ACT
                    if h % 2 == 0:
                        nc.vector.tensor_copy(y3[:, h, :], py[:])
                    else:
                        nc.scalar.mul(y3[:, h, :], py[:], 1.0)
                    # y write-out on the gpsimd ring (idle mid-stream); the
                    # final h goes on the scalar queue right after its own
                    # copy so the very last transfer is small and immediate
                    if last_e and h == HT - 2:
                        nc.gpsimd.dma_start(yt_d[:, h, :], y3[:, h, :])
                    elif h % 2 == 1:
                        nc.gpsimd.dma_start(
                            yt_d[:, h - 1 : h + 1, :], y3[:, h - 1 : h + 1, :]
                        )

    nc.finalize()
    _BUILD_CACHE[key] = nc
    return nc


def _build_bf16(S):
    """bf16 fallback: identical math with K=128 matmuls (previous baseline)."""
    import concourse.bacc as bacc
    import concourse.bass as bass
    import concourse.mybir as mybir
    from concourse import tile

    key = (S, "bf16")
    if key in _BUILD_CACHE:
        return _BUILD_CACHE[key]

    FT = mybir.dt.float32
    BF = mybir.dt.bfloat16
    silu_fn = mybir.ActivationFunctionType.Silu

    chunks = _chunks(S)

    nc = bacc.Bacc(None)
    xt_d = nc.declare_dram_parameter("xt", [E_PER_CORE, 128, HT, S], BF, isOutput=False)
    gu_d = nc.declare_dram_parameter("guw", [E_PER_CORE, 128, HT, 1024], BF, isOutput=False)
    dn_d = nc.declare_dram_parameter("dnw", [E_PER_CORE, 128, IT, 1024], BF, isOutput=False)
    yt_d = nc.declare_dram_parameter("yt", [E_PER_CORE, 128, HT, S], BF, isOutput=True)

    with tile.TileContext(nc) as tc:
        with (
            tc.tile_pool(name="xpool", bufs=2) as xpool,
            tc.tile_pool(name="gupool", bufs=2) as gupool,
            tc.tile_pool(name="dnpool", bufs=2) as dnpool,
            tc.tile_pool(name="midpool", bufs=2) as midpool,
            tc.tile_pool(name="silpool", bufs=E_PER_CORE * IT * len(chunks)) as silpool,
            tc.tile_pool(name="ypool", bufs=2) as ypool,
            tc.tile_pool(name="pgpool", bufs=3, space="PSUM") as pgpool,
            tc.tile_pool(name="pupool", bufs=3, space="PSUM") as pupool,
            tc.tile_pool(name="pypool", bufs=2, space="PSUM") as pypool,
        ):
            for e in range(E_PER_CORE):
                xe = xpool.tile([128, HT, S], BF, tag="xt")
                ge = gupool.tile([128, HT, 1024], BF, tag="gu")
                de = dnpool.tile([128, IT, 1024], BF, tag="dn")
                for h in range(HT):
                    nc.sync.dma_start(ge[:, h, :], gu_d[e, :, h, :])
                    nc.scalar.dma_start(xe[:, h, :], xt_d[e, :, h, :])
                for i in range(IT):
                    nc.sync.dma_start(de[:, i, :], dn_d[e, :, i, :])

                mid = midpool.tile([128, IT, S], BF, tag="mid")
                for ci, (c0, cn) in enumerate(chunks):
                    for oi in range(IT):
                        pg = pgpool.tile([128, cn], FT, tag="pg")
                        pu = pupool.tile([128, cn], FT, tag="pu")
                        for h in range(HT):
                            nc.tensor.matmul(
                                pg[:],
                                ge[:, h, oi * 128 : (oi + 1) * 128],
                                xe[:, h, c0 : c0 + cn],
                                start=(h == 0),
                                stop=(h == HT - 1),
                            )
                        for h in range(HT):
                            nc.tensor.matmul(
                                pu[:],
                                ge[:, h, 512 + oi * 128 : 512 + (oi + 1) * 128],
                                xe[:, h, c0 : c0 + cn],
                                start=(h == 0),
                                stop=(h == HT - 1),
                            )
                        sil = silpool.tile([128, cn], FT, tag="sil")
                        nc.scalar.activation(sil[:], pg[:], silu_fn)
                        nc.vector.scalar_tensor_tensor(
                            mid[:, oi, c0 : c0 + cn], pu[:], 1.0, sil[:],
                            mybir.AluOpType.mult, mybir.AluOpType.mult,
                        )
                    y3 = ypool.tile([128, HT, S], BF, tag="yo") if ci == 0 else y3
                    for h in range(HT):
                        py = pypool.tile([128, cn], FT, tag="py")
                        for i in range(IT):
                            nc.tensor.matmul(
                                py[:],
                                de[:, i, h * 128 : (h + 1) * 128],
                                mid[:, i, c0 : c0 + cn],
                                start=(i == 0),
                                stop=(i == IT - 1),
                            )
                        if h % 2 == 0:
                            nc.vector.tensor_copy(y3[:, h, c0 : c0 + cn], py[:])
                        else:
                            nc.scalar.mul(y3[:, h, c0 : c0 + cn], py[:], 1.0)
                            if ci == len(chunks) - 1:
                                nc.gpsimd.dma_start(
                                    yt_d[e, :, h - 1 : h + 1, :], y3[:, h - 1 : h + 1, :]
                                )

    nc.finalize()
    _BUILD_CACHE[key] = nc
    return nc


def _install_trace_shims():
    """Make trace=True usable in this image: provide the NTFF hook module and
    neutralize the artifact upload (no bucket access needed for local use)."""
    import sys
    import types

    try:
        import antenv.axon_hooks  # noqa: F401
    except ImportError:
        hook = None
        try:
            from trn_agent_boot.trn_boot import _ntff_profile_via_ctypes

            hook = _ntff_profile_via_ctypes("/opt/axon/libaxon_pjrt.so")
        except Exception:
            hook = None
        mod = types.ModuleType("antenv.axon_hooks")
        mod._hook = hook
        mod.get_axon_ntff_profile_hook = lambda: mod._hook
        mod.set_axon_ntff_profile_hook = lambda h: setattr(mod, "_hook", h)
        sys.modules["antenv.axon_hooks"] = mod

    import concourse.bass_utils as bu

    orig_upload = bu.upload_artifacts

    def safe_upload(tmpdir):
        try:
            return orig_upload(tmpdir)
        except Exception:
            return tmpdir
    bu.upload_artifacts = safe_upload


def kernel(**inputs):
    import ml_dtypes
    from concourse.bass_utils import run_bass_kernel_spmd

    hidden = np.ascontiguousarray(np.asarray(inputs["hidden_states"], dtype=np.float32))
    idx = np.asarray(inputs["top_k_index"]).astype(np.int64)
    wts = np.asarray(inputs["top_k_weights"], dtype=np.float32)
    gup = np.asarray(inputs["gate_up_proj"], dtype=np.float32)
    dnp = np.asarray(inputs["down_proj"], dtype=np.float32)

    n_tok = hidden.shape[0]
    K = idx.shape[1]

    ve, vt, vw, vp, va, zero_w = _route(idx, wts, n_tok)
    cnts = np.bincount(ve, minlength=R)
    maxc = int(cnts.max())
    # N multiple of 64 elements keeps the PE moving-operand stream at full rate
    S = max(256, ((maxc + 63) // 64) * 64)

    # the fp8 pipeline is single-chunk (S <= 512 per PSUM bank); a pathological
    # routing beyond that falls back to the chunked bf16 pipeline
    prec = PREC if S <= 512 else "bf16"

    if prec == "fp8":
        io_np = ml_dtypes.float8_e4m3  # TRN FP8_EXP4: max +-240, IEEE-style
        wmul = WSCALE
    else:
        io_np = ml_dtypes.bfloat16
        wmul = 1.0

    estarts = np.cumsum(cnts) - cnts

    if prec == "fp8":
        # rank experts by count (desc); slot k on core c takes rank k*8+c, so
        # slot k's free dim only covers the k-th-octile counts (rounded to 32
        # to keep the PE moving-operand stream at full rate)
        order = np.argsort(-cnts, kind="stable")
        assign = order.reshape(E_PER_CORE, N_CORES)  # [slot, core]
        SL = tuple(
            max(256, int(np.ceil(cnts[assign[k]].max() / 32)) * 32)
            for k in range(E_PER_CORE)
        )
    else:
        assign = np.arange(R).reshape(N_CORES, E_PER_CORE).T  # contiguous
        SL = tuple(S for _ in range(E_PER_CORE))

    def pack_w(w, experts, kt):
        # [n, out, in] -> [n, 128, kt, out] tiles of the transposed weight
        return np.ascontiguousarray(
            (w[experts] * wmul)
            .transpose(0, 2, 1)
            .reshape(len(experts), kt, 128, w.shape[1])
            .transpose(0, 2, 1, 3)
            .astype(io_np)
        )

    in_maps = []
    for c in range(N_CORES):
        experts = [int(assign[k][c]) for k in range(E_PER_CORE)]
        m = {
            "guw": pack_w(gup, experts, HT),
            "dnw": pack_w(dnp, experts, IT),
        }
        xts = []
        for k, ge in enumerate(experts):
            xt = np.zeros((128, HT, SL[k]), dtype=io_np)
            s0, cnt = estarts[ge], cnts[ge]
            if cnt:
                toks = vt[s0 : s0 + cnt]
                # [cnt, H] -> [H, cnt] -> [HT, 128, cnt] -> [128, HT, cnt]
                xt[:, :, :cnt] = (
                    hidden[toks].T.reshape(HT, 128, cnt).transpose(1, 0, 2)
                ).astype(io_np)
            xts.append(xt)
        if prec == "fp8":
            for k in range(E_PER_CORE):
                m[f"xt{k}"] = xts[k]
        else:
            m["xt"] = np.ascontiguousarray(np.stack(xts))
        in_maps.append(m)

    nc = _build_fp8(SL) if prec == "fp8" else _build_bf16(S)

    trace = bool(int(os.environ.get("KERNEL_TRACE", "0")))
    # always shim: harmless when tracing is off, and keeps the trace path
    # alive if the caller enables BASS_TRACE without our env var
    try:
        _install_trace_shims()
    except Exception:
        pass
    res = run_bass_kernel_spmd(nc, in_maps, list(range(N_CORES)), trace=trace)
    LAST_RUN["exec_time_ns"] = res.exec_time_ns
    LAST_RUN["mean_exec_time_ns"] = res.mean_exec_time_ns
    LAST_RUN["instructions_and_trace"] = res.instructions_and_trace
    LAST_RUN["profile_json"] = res.profile_json

    # ---- combine on host ----
    yscale = 1.0 / WSCALE if prec == "fp8" else 1.0
    out = hidden * zero_w[:, None].astype(np.float32)
    acc = np.zeros((n_tok * K, H), dtype=np.float32)
    for c in range(N_CORES):
        for k in range(E_PER_CORE):
            ge = int(assign[k][c])
            s0, cnt = estarts[ge], cnts[ge]
            if cnt == 0:
                continue
            if prec == "fp8":
                yt = np.asarray(res.results[c][f"yt{k}"]).astype(np.float32)
            else:
                yt = np.asarray(res.results[c]["yt"][k]).astype(np.float32)
            # [128, HT, Sk] -> [HT, 128, Sk] -> [H, Sk]
            y = yt.transpose(1, 0, 2).reshape(H, -1)[:, :cnt].T
            acc[va[s0 : s0 + cnt]] = y * (vw[s0 : s0 + cnt, None] * yscale)
    out += acc.reshape(n_tok, K, H).sum(axis=1)
    return out


# revision 58
# speedup vs baseline: 1.0037x; 1.0037x over previous
"""Trainium2 Bass kernel for LongcatFlash MoE experts (expert-parallel, 8 cores).

Problem: T=4096 tokens, H=1024, I=512, 32 routed + 8 zero (identity) experts,
top-4 routing, per-expert capacity 768.

Strategy (sharding_hint = expert parallelism):
  - Host: compute routing (stable sort by expert, capacity clip), permute
    tokens to their expert's core (the "all-to-all"), build per-core packed
    activation buffers with tokens on the GEMM free dimension.
  - Device (8 cores, SPMD): each core owns 4 routed experts; per expert run
    the gated MLP as tiled matmuls with tokens on the free dim:
        gu[o, c]  = sum_h guT[h, o] * xT[h, c]      (o = 2I rows, c = tokens)
        mid[i, c] = silu(gate[i, c]) * up[i, c]
        y[h, c]   = sum_i dnT[i, h] * mid[i, c]
  - Host: gather per-assignment outputs, scale by router weight, scatter-add
    back per token, add the zero-expert weighted-identity term.

Precision modes:
  - "fp8" (default): e4m3 weights/activations, DoubleRow matmuls (K=256 per
    MM, 2 fp8 MACs per PE cell per cycle).  Weights are pre-scaled by 128 on
    the host to sit in e4m3's precision sweet spot; the 1/128 is folded into
    the on-device silu/up scaling and the host-side combine.  TRN's e4m3 has
    max +-240 (= ml_dtypes.float8_e4m3, not the OCP "fn" variant).
  - "bf16": fallback, plain K=128 matmuls.
"""

import math
import os

import numpy as np

N_CORES = 8
R = 32  # routed experts
E_PER_CORE = R // N_CORES  # 4
CAPACITY = 768
H = 1024
I_DIM = 512
HT = H // 128  # 8 h-tiles
OT = 2 * I_DIM // 128  # 8 o-tiles of gate_up
IT = I_DIM // 128  # 4 i-tiles

WSCALE = 128.0  # fp8 weight pre-scale (power of 2: exact to undo)

PREC = os.environ.get("MOE_PREC", "fp8")

LAST_RUN = {}  # filled with exec_time_ns etc. for test harness use


def _route(idx, wts, n_tok):
    """Replicates the reference's capacity-buffer routing exactly.

    Returns per-assignment (expert, token, weight, slot, flat_index) for kept
    routed assignments, sorted by expert (stable), plus zero-expert weights.
    """
    K = idx.shape[1]
    A = n_tok * K
    flat_e = idx.reshape(-1).astype(np.int64)
    flat_t = np.repeat(np.arange(n_tok, dtype=np.int64), K)
    flat_w = wts.reshape(-1)
    order = np.argsort(flat_e, kind="stable")
    se = flat_e[order]
    st = flat_t[order]
    sw = flat_w[order]
    counts = np.bincount(flat_e, minlength=R + 8)
    starts = np.cumsum(counts) - counts
    pos = np.arange(A, dtype=np.int64) - starts[se]
    valid = (se < R) & (pos < CAPACITY)
    zero_w = np.where(idx >= R, wts, 0.0).sum(axis=1)
    return (
        se[valid],
        st[valid],
        sw[valid],
        pos[valid],
        order[valid],
        zero_w,
    )


def _chunks(S):
    n = (S + 511) // 512
    base = S // n
    rem = S - base * n
    out = []
    c0 = 0
    for i in range(n):
        cn = base + (1 if i < rem else 0)
        out.append((c0, cn))
        c0 += cn
    return out


_BUILD_CACHE = {}


def _build_fp8(SL):
    """fp8 e4m3 DoubleRow pipeline: per expert-slot
      4 gate psums + 4 up psums (4 DoubleRow MMs each, K=256),
      silu+mult -> fp8 mid, 8 down psums (2 DoubleRow MMs each),
      copy -> bf16 y, DMA out per h-pair.

    SL is a tuple of per-slot free-dim sizes (descending).  Every core runs
    the same instruction stream, but expert-to-slot assignment is per-core
    (rank k*8+c goes to core c slot k), so slot k only needs to cover the
    k-th largest per-core expert count.
    """
    import concourse.bacc as bacc
    import concourse.bass as bass
    import concourse.mybir as mybir
    from concourse import tile

    key = (tuple(SL), "fp8")
    if key in _BUILD_CACHE:
        return _BUILD_CACHE[key]

    FT = mybir.dt.float32
    F8 = mybir.dt.float8e4
    BF = mybir.dt.bfloat16
    DR = mybir.MatmulPerfMode.DoubleRow
    silu_fn = mybir.ActivationFunctionType.Silu

    assert all(256 <= s <= 512 for s in SL), "fp8 path assumes 256 <= S <= 512"

    nc = bacc.Bacc(None)
    xts = [
        nc.declare_dram_parameter(f"xt{k}", [128, HT, SL[k]], F8, isOutput=False)
        for k in range(E_PER_CORE)
    ]
    gu_d = nc.declare_dram_parameter("guw", [E_PER_CORE, 128, HT, 1024], F8, isOutput=False)
    dn_d = nc.declare_dram_parameter("dnw", [E_PER_CORE, 128, IT, 1024], F8, isOutput=False)
    yts = [
        nc.declare_dram_parameter(f"yt{k}", [128, HT, SL[k]], BF, isOutput=True)
        for k in range(E_PER_CORE)
    ]

    inv = 1.0 / WSCALE

    with tile.TileContext(nc) as tc:
        with (
            tc.tile_pool(name="xpool", bufs=3) as xpool,
            tc.tile_pool(name="gupool", bufs=3) as gupool,
            tc.tile_pool(name="dnpool", bufs=3) as dnpool,
            tc.tile_pool(name="midpool", bufs=2) as midpool,
            # sil tiles are ACT-written; unique slots (no reuse) keep the
            # Activation instruction at a single sync-wait (AC struct limit 1)
            tc.tile_pool(name="silpool", bufs=E_PER_CORE * IT) as silpool,
            tc.tile_pool(name="ypool", bufs=3) as ypool,
            tc.tile_pool(name="pgpool", bufs=2, space="PSUM") as pgpool,
            tc.tile_pool(name="pupool", bufs=2, space="PSUM") as pupool,
            tc.tile_pool(name="pypool", bufs=4, space="PSUM") as pypool,
        ):
            # Warm the PE clock gate (HAM) with dummy matmuls during the
            # initial DMA wait: ~3us of sustained PE activity flips the clock
            # from 1.2 to 2.4 GHz, so the real stream starts at full rate.
            wz = ypool.tile([128, 2, 256], F8, tag="warm")
            nc.vector.memset(wz[:], 0)
            pd = pypool.tile([128, SL[0]], FT, tag="py")
            for _ in range(16):
                nc.tensor.matmul(
                    pd[:, 0:256], wz[:, :, 0:128], wz[:], start=True, stop=True,
                    perf_mode=DR,
                )
            # Just-in-time per-expert h-pair loads (upfront whole-tensor
            # prefetch saturates the DMA fabric and starves the critical
            # expert-0 pieces).  gu/dn ride the sync (SP HWDGE) ring, x the
            # scalar (ACT HWDGE) ring.  Expert 0's first gu pair is split
            # gate-half/up-half so the first matmul unblocks earlier.
            for e in range(E_PER_CORE):
                Se = SL[e]
                xt_d = xts[e]
                yt_d = yts[e]
                xe = xpool.tile([128, HT, Se], F8, tag="xt")
                ge = gupool.tile([128, HT, 1024], F8, tag="gu")
                de = dnpool.tile([128, IT, 1024], F8, tag="dn")
                # expert 0 is latency-critical: fine pieces so the first MMs
                # unblock ASAP, dn0 on the scalar ring right after x0 (ready
                # ~13us, needed ~17).  Experts 1-3 are prefetched a full
                # phase ahead, so coarse 512KB transfers (fewer ~1.3us HWDGE
                # ring round trips, better DMA efficiency) win there.
                if e == 0:
                    # 10 pieces, need-times ~10 + 1.3*slot us, 2 HWDGE rings =
                    # 2 slots per 1.3us.  Pair 1's two pieces both need slot
                    # <=2.5 but only one #2 slot is free, so ONE tiny piece
                    # (x pair1, 114KB) rides the gpsimd ring — small enough
                    # that the 3-ring bandwidth tax (~0.3us) stays below the
                    # 0.65us stall it removes (v19/v22 moved 0.5-0.7MB there
                    # and lost).  g1 then takes scalar slot 2.
                    nc.sync.dma_start(ge[:, 0:2, 0:512], gu_d[0, :, 0:2, 0:512])
                    nc.scalar.dma_start(xe[:, 0:2, :], xt_d[:, 0:2, :])
                    nc.gpsimd.dma_start(xe[:, 2:4, :], xt_d[:, 2:4, :])
                    nc.sync.dma_start(ge[:, 0:2, 512:1024], gu_d[0, :, 0:2, 512:1024])
                    nc.scalar.dma_start(ge[:, 2:4, :], gu_d[0, :, 2:4, :])
                    nc.sync.dma_start(ge[:, 4:6, :], gu_d[0, :, 4:6, :])
                    nc.scalar.dma_start(xe[:, 4:6, :], xt_d[:, 4:6, :])
                    nc.sync.dma_start(ge[:, 6:8, :], gu_d[0, :, 6:8, :])
                    nc.scalar.dma_start(xe[:, 6:8, :], xt_d[:, 6:8, :])
                    for ii in range(IT // 2):
                        nc.scalar.dma_start(
                            de[:, 2 * ii : 2 * ii + 2, :],
                            dn_d[0, :, 2 * ii : 2 * ii + 2, :],
                        )
                else:
                    # all on the sync ring: its FIFO is program order, so the
                    # loads self-throttle behind expert-0's critical pieces,
                    # and the scalar queue stays compute-only (x triggers were
                    # firing ~1us late behind the previous expert's silu/muls)
                    nc.sync.dma_start(ge[:, 0:4, :], gu_d[e, :, 0:4, :])
                    nc.sync.dma_start(xe[:], xt_d[:])
                    nc.sync.dma_start(ge[:, 4:HT, :], gu_d[e, :, 4:HT, :])
                    nc.sync.dma_start(de[:], dn_d[e])

                mid = midpool.tile([128, IT, Se], F8, tag="mid")
                for oi in range(IT):
                    pg = pgpool.tile([128, Se], FT, tag="pg")
                    pu = pupool.tile([128, Se], FT, tag="pu")
                    for hh in range(HT // 2):
                        nc.tensor.matmul(
                            pg[:],
                            ge[:, 2 * hh : 2 * hh + 2, oi * 128 : (oi + 1) * 128],
                            xe[:, 2 * hh : 2 * hh + 2, :],
                            start=(hh == 0),
                            stop=(hh == HT // 2 - 1),
                            perf_mode=DR,
                        )
                    for hh in range(HT // 2):
                        nc.tensor.matmul(
                            pu[:],
                            ge[:, 2 * hh : 2 * hh + 2, 512 + oi * 128 : 512 + (oi + 1) * 128],
                            xe[:, 2 * hh : 2 * hh + 2, :],
                            start=(hh == 0),
                            stop=(hh == HT // 2 - 1),
                            perf_mode=DR,
                        )
                    sil = silpool.tile([128, Se], FT, tag="sil")
                    nc.scalar.activation(sil[:], pg[:], silu_fn, scale=inv)
                    nc.vector.scalar_tensor_tensor(
                        mid[:, oi, :], pu[:], inv, sil[:],
                        mybir.AluOpType.mult, mybir.AluOpType.mult,
                    )

                y3 = ypool.tile([128, HT, Se], BF, tag="yo")
                last_e = e == E_PER_CORE - 1
                for h in range(HT):
                    if last_e and h == HT - 1:
                        # pipeline the tail: the final h-tile runs as two
                        # free-dim chunks so copy/DMA of chunk 0 overlaps the
                        # matmuls of chunk 1, and the last transfer is small
                        for c0, cn in ((0, 256), (256, Se - 256)):
                            py = pypool.tile([128, cn], FT, tag="py")
                            for ii in range(IT // 2):
                                nc.tensor.matmul(
                                    py[:],
                                    de[:, 2 * ii : 2 * ii + 2, h * 128 : (h + 1) * 128],
                                    mid[:, 2 * ii : 2 * ii + 2, c0 : c0 + cn],
                                    start=(ii == 0),
                                    stop=(ii == IT // 2 - 1),
                                    perf_mode=DR,
                                )
                            if c0 == 0:
                                nc.vector.tensor_copy(y3[:, h, c0 : c0 + cn], py[:])
                                # sync ring is idle at the tail; gpsimd's Q7 is
                                # still serializing the h1-h6 write-out triggers
                                nc.sync.dma_start(
                                    yt_d[:, h, c0 : c0 + cn], y3[:, h, c0 : c0 + cn]
                                )
                            else:
                                nc.scalar.mul(y3[:, h, c0 : c0 + cn], py[:], 1.0)
                                nc.scalar.dma_start(
                                    yt_d[:, h, c0 : c0 + cn], y3[:, h, c0 : c0 + cn]
                                )
                        continue
                    py = pypool.tile([128, Se], FT, tag="py")
                    for ii in range(IT // 2):
                        nc.tensor.matmul(
                            py[:],
                            de[:, 2 * ii : 2 * ii + 2, h * 128 : (h + 1) * 128],
                            mid[:, 2 * ii : 2 * ii + 2, :],
                            start=(ii == 0),
                            stop=(ii == IT // 2 - 1),
                            perf_mode=DR,
                        )
                    # split psum->bf16 copies across DVE and ACT
                    if h % 2 == 0:
                        nc.vector.tensor_copy(y3[:, h, :], py[:])
                    else:
                        nc.scalar.mul(y3[:, h, :], py[:], 1.0)
                    # y write-out on the gpsimd ring (idle mid-stream); the
                    # final h goes on the scalar queue right after its own
                    # copy so the very last transfer is small and immediate
                    if last_e and h == HT - 2:
                        nc.gpsimd.dma_start(yt_d[:, h, :], y3[:, h, :])
                    elif h % 2 == 1:
                        nc.gpsimd.dma_start(
                            yt_d[:, h - 1 : h + 1, :], y3[:, h - 1 : h + 1, :]
                        )

    nc.finalize()
    _BUILD_CACHE[key] = nc
    return nc


def _build_bf16(S):
    """bf16 fallback: identical math with K=128 matmuls (previous baseline)."""
    import concourse.bacc as bacc
    import concourse.bass as bass
    import concourse.mybir as mybir
    from concourse import tile

    key = (S, "bf16")
    if key in _BUILD_CACHE:
        return _BUILD_CACHE[key]

    FT = mybir.dt.float32
    BF = mybir.dt.bfloat16
    silu_fn = mybir.ActivationFunctionType.Silu

    chunks = _chunks(S)

    nc = bacc.Bacc(None)
    xt_d = nc.declare_dram_parameter("xt", [E_PER_CORE, 128, HT, S], BF, isOutput=False)
    gu_d = nc.declare_dram_parameter("guw", [E_PER_CORE, 128, HT, 1024], BF, isOutput=False)
    dn_d = nc.declare_dram_parameter("dnw", [E_PER_CORE, 128, IT, 1024], BF, isOutput=False)
    yt_d = nc.declare_dram_parameter("yt", [E_PER_CORE, 128, HT, S], BF, isOutput=True)

    with tile.TileContext(nc) as tc:
        with (
            tc.tile_pool(name="xpool", bufs=2) as xpool,
            tc.tile_pool(name="gupool", bufs=2) as gupool,
            tc.tile_pool(name="dnpool", bufs=2) as dnpool,
            tc.tile_pool(name="midpool", bufs=2) as midpool,
            tc.tile_pool(name="silpool", bufs=E_PER_CORE * IT * len(chunks)) as silpool,
            tc.tile_pool(name="ypool", bufs=2) as ypool,
            tc.tile_pool(name="pgpool", bufs=3, space="PSUM") as pgpool,
            tc.tile_pool(name="pupool", bufs=3, space="PSUM") as pupool,
            tc.tile_pool(name="pypool", bufs=2, space="PSUM") as pypool,
        ):
            for e in range(E_PER_CORE):
                xe = xpool.tile([128, HT, S], BF, tag="xt")
                ge = gupool.tile([128, HT, 1024], BF, tag="gu")
                de = dnpool.tile([128, IT, 1024], BF, tag="dn")
                for h in range(HT):
                    nc.sync.dma_start(ge[:, h, :], gu_d[e, :, h, :])
                    nc.scalar.dma_start(xe[:, h, :], xt_d[e, :, h, :])
                for i in range(IT):
                    nc.sync.dma_start(de[:, i, :], dn_d[e, :, i, :])

                mid = midpool.tile([128, IT, S], BF, tag="mid")
                for ci, (c0, cn) in enumerate(chunks):
                    for oi in range(IT):
                        pg = pgpool.tile([128, cn], FT, tag="pg")
                        pu = pupool.tile([128, cn], FT, tag="pu")
                        for h in range(HT):
                            nc.tensor.matmul(
                                pg[:],
                                ge[:, h, oi * 128 : (oi + 1) * 128],
                                xe[:, h, c0 : c0 + cn],
                                start=(h == 0),
                                stop=(h == HT - 1),
                            )
                        for h in range(HT):
                            nc.tensor.matmul(
                                pu[:],
                                ge[:, h, 512 + oi * 128 : 512 + (oi + 1) * 128],
                                xe[:, h, c0 : c0 + cn],
                                start=(h == 0),
                                stop=(h == HT - 1),
                            )
                        sil = silpool.tile([128, cn], FT, tag="sil")
                        nc.scalar.activation(sil[:], pg[:], silu_fn)
                        nc.vector.scalar_tensor_tensor(
                            mid[:, oi, c0 : c0 + cn], pu[:], 1.0, sil[:],
                            mybir.AluOpType.mult, mybir.AluOpType.mult,
                        )
                    y3 = ypool.tile([128, HT, S], BF, tag="yo") if ci == 0 else y3
                    for h in range(HT):
                        py = pypool.tile([128, cn], FT, tag="py")
                        for i in range(IT):
                            nc.tensor.matmul(
                                py[:],
                                de[:, i, h * 128 : (h + 1) * 128],
                                mid[:, i, c0 : c0 + cn],
                                start=(i == 0),
                                stop=(i == IT - 1),
                            )
                        if h % 2 == 0:
                            nc.vector.tensor_copy(y3[:, h, c0 : c0 + cn], py[:])
                        else:
                            nc.scalar.mul(y3[:, h, c0 : c0 + cn], py[:], 1.0)
                            if ci == len(chunks) - 1:
                                nc.gpsimd.dma_start(
                                    yt_d[e, :, h - 1 : h + 1, :], y3[:, h - 1 : h + 1, :]
                                )

    nc.finalize()
    _BUILD_CACHE[key] = nc
    return nc


def _install_trace_shims():
    """Make trace=True usable in this image: provide the NTFF hook module and
    neutralize the artifact upload (no bucket access needed for local use)."""
    import sys
    import types

    try:
        import antenv.axon_hooks  # noqa: F401
    except ImportError:
        hook = None
        try:
            from trn_agent_boot.trn_boot import _ntff_profile_via_ctypes

            hook = _ntff_profile_via_ctypes("/opt/axon/libaxon_pjrt.so")
        except Exception:
            hook = None
        mod = types.ModuleType("antenv.axon_hooks")
        mod._hook = hook
        mod.get_axon_ntff_profile_hook = lambda: mod._hook
        mod.set_axon_ntff_profile_hook = lambda h: setattr(mod, "_hook", h)
        sys.modules["antenv.axon_hooks"] = mod

    import concourse.bass_utils as bu

    orig_upload = bu.upload_artifacts

    def safe_upload(tmpdir):
        try:
            return orig_upload(tmpdir)
        except Exception:
            return tmpdir
    bu.upload_artifacts = safe_upload


def kernel(**inputs):
    import ml_dtypes
    from concourse.bass_utils import run_bass_kernel_spmd

    hidden = np.ascontiguousarray(np.asarray(inputs["hidden_states"], dtype=np.float32))
    idx = np.asarray(inputs["top_k_index"]).astype(np.int64)
    wts = np.asarray(inputs["top_k_weights"], dtype=np.float32)
    gup = np.asarray(inputs["gate_up_proj"], dtype=np.float32)
    dnp = np.asarray(inputs["down_proj"], dtype=np.float32)

    n_tok = hidden.shape[0]
    K = idx.shape[1]

    ve, vt, vw, vp, va, zero_w = _route(idx, wts, n_tok)
    cnts = np.bincount(ve, minlength=R)
    maxc = int(cnts.max())
    # N multiple of 64 elements keeps the PE moving-operand stream at full rate
    S = max(256, ((maxc + 63) // 64) * 64)

    # the fp8 pipeline is single-chunk (S <= 512 per PSUM bank); a pathological
    # routing beyond that falls back to the chunked bf16 pipeline
    prec = PREC if S <= 512 else "bf16"

    if prec == "fp8":
        io_np = ml_dtypes.float8_e4m3  # TRN FP8_EXP4: max +-240, IEEE-style
        wmul = WSCALE
    else:
        io_np = ml_dtypes.bfloat16
        wmul = 1.0

    estarts = np.cumsum(cnts) - cnts

    if prec == "fp8":
        # rank experts by count (desc); slot k on core c takes rank k*8+c, so
        # slot k's free dim only covers the k-th-octile counts (rounded to 32
        # to keep the PE moving-operand stream at full rate)
        order = np.argsort(-cnts, kind="stable")
        assign = order.reshape(E_PER_CORE, N_CORES)  # [slot, core]
        SL = tuple(
            max(256, int(np.ceil(cnts[assign[k]].max() / 32)) * 32)
            for k in range(E_PER_CORE)
        )
    else:
        assign = np.arange(R).reshape(N_CORES, E_PER_CORE).T  # contiguous
        SL = tuple(S for _ in range(E_PER_CORE))

    def pack_w(w, experts, kt):
        # [n, out, in] -> [n, 128, kt, out] tiles of the transposed weight
        return np.ascontiguousarray(
            (w[experts] * wmul)
            .transpose(0, 2, 1)
            .reshape(len(experts), kt, 128, w.shape[1])
            .transpose(0, 2, 1, 3)
            .astype(io_np)
        )

    in_maps = []
    for c in range(N_CORES):
        experts = [int(assign[k][c]) for k in range(E_PER_CORE)]
        m = {
            "guw": pack_w(gup, experts, HT),
            "dnw": pack_w(dnp, experts, IT),
        }
        xts = []
        for k, ge in enumerate(experts):
            xt = np.zeros((128, HT, SL[k]), dtype=io_np)
            s0, cnt = estarts[ge], cnts[ge]
            if cnt:
                toks = vt[s0 : s0 + cnt]
                # [cnt, H] -> [H, cnt] -> [HT, 128, cnt] -> [128, HT, cnt]
                xt[:, :, :cnt] = (
                    hidden[toks].T.reshape(HT, 128, cnt).transpose(1, 0, 2)
                ).astype(io_np)
            xts.append(xt)
        if prec == "fp8":
            for k in range(E_PER_CORE):
                m[f"xt{k}"] = xts[k]
        else:
            m["xt"] = np.ascontiguousarray(np.stack(xts))
        in_maps.append(m)

    nc = _build_fp8(SL) if prec == "fp8" else _build_bf16(S)

    trace = bool(int(os.environ.get("KERNEL_TRACE", "0")))
    # always shim: harmless when tracing is off, and keeps the trace path
    # alive if the caller enables BASS_TRACE without our env var
    try:
        _install_trace_shims()
    except Exception:
        pass
    res = run_bass_kernel_spmd(nc, in_maps, list(range(N_CORES)), trace=trace)
    LAST_RUN["exec_time_ns"] = res.exec_time_ns
    LAST_RUN["mean_exec_time_ns"] = res.mean_exec_time_ns
    LAST_RUN["instructions_and_trace"] = res.instructions_and_trace
    LAST_RUN["profile_json"] = res.profile_json

    # ---- combine on host ----
    yscale = 1.0 / WSCALE if prec == "fp8" else 1.0
    out = hidden * zero_w[:, None].astype(np.float32)
    acc = np.zeros((n_tok * K, H), dtype=np.float32)
    for c in range(N_CORES):
        for k in range(E_PER_CORE):
            ge = int(assign[k][c])
            s0, cnt = estarts[ge], cnts[ge]
            if cnt == 0:
                continue
            if prec == "fp8":
                yt = np.asarray(res.results[c][f"yt{k}"]).astype(np.float32)
            else:
                yt = np.asarray(res.results[c]["yt"][k]).astype(np.float32)
            # [128, HT, Sk] -> [HT, 128, Sk] -> [H, Sk]
            y = yt.transpose(1, 0, 2).reshape(H, -1)[:, :cnt].T
            acc[va[s0 : s0 + cnt]] = y * (vw[s0 : s0 + cnt, None] * yscale)
    out += acc.reshape(n_tok, K, H).sum(axis=1)
    return out
